# revision 9
# baseline (speedup 1.0000x reference)
"""bf16 structural variant: single AllGather for G with pipelined per-k-tile
reload, partition-major C AllReduce with the r row folded into a 9th k-tile
consumed by an augmented-xts phase-5 matmul, weight loads kept off the
phase-1 DMA stream, and an engine-balanced LN/FFN tail.

Math (head h on core h), with xa = [x | 1], G~ = xa^T xa (symmetric):
  scores_h = Wka_h G~ Wqa_h^T / sqrt(E);  A_h = softmax(scores_h)
  D_h = A_h^T [Wv_h | bv_h];  C_h = D_h^T Wz_h^T (+ r row from D[:,E], bz/8)
  O = xa_s [Csum; rsum];  LN1 = ln(O)+x;  FN = LN1 Wf^T + bf; out = ln(FN)+LN1
"""

import numpy as np
import ml_dtypes

import concourse.bass as bass
import concourse.mybir as mybir
import concourse.tile as tile
from concourse import bacc
from concourse.bass_utils import run_bass_kernel_spmd
from concourse.masks import make_identity

S, E, H = 4096, 1024, 8
P = 128
EA = 1152          # augmented (E + ones row) padded to 9*128 (weights only)
GW = E + 8         # G col width: E cols + ones col at E, padded to 1032
NET = E // P       # 8
NAT = EA // P      # 9
NKT = S // P       # 32 sequence tiles (full S)
SS = S // H        # 512 rows per core (contiguous shard)
NSS = SS // P      # 4
NH = 2             # 512-wide free-dim halves of E
WVW = E + 8        # wva width: Wv cols + bv col at E, padded to 1032
DW = 1152          # d_sb free width: WVW padded so m-tile 8 is 128 wide
CW = 9 * 512       # c_part cols per half: 8 C k-tiles + r tile, part-major
EPS = 1e-5
SCALE = 1.0 / 32.0  # 1/sqrt(E)

F32 = mybir.dt.float32
BF16 = mybir.dt.bfloat16

L_G1, L_B1, L_BF, L_G2, L_B2 = range(5)

LAST_RESULT = None


def _bcast_row(t: bass.AP) -> bass.AP:
    return bass.AP(tensor=t.tensor, offset=t.offset, ap=[[0, P], [1, t.shape[-1]]])


def build_nc(id_g1b1=False, id_g2b2=False):
    nc = bacc.Bacc(num_devices=H)

    xap = nc.declare_dram_parameter("xap", [P, NKT * GW], BF16, isOutput=False)
    xcp = nc.declare_dram_parameter("xcp", [P, NKT * P], BF16, isOutput=False)
    wqa = nc.declare_dram_parameter("wqa", [P, NAT * E], BF16, isOutput=False)
    wka = nc.declare_dram_parameter("wka", [P, NAT * E], BF16, isOutput=False)
    wva = nc.declare_dram_parameter("wva", [P, NET * WVW], BF16, isOutput=False)
    wzT = nc.declare_dram_parameter("wzT", [P, NET * E], BF16, isOutput=False)
    wfT = nc.declare_dram_parameter("wfT", [P, NET * E], BF16, isOutput=False)
    xts = nc.declare_dram_parameter("xts", [P, NAT * SS], BF16, isOutput=False)
    xs = nc.declare_dram_parameter("xs", [SS, E], BF16, isOutput=False)
    rows = nc.declare_dram_parameter("rows", [6, E], F32, isOutput=False)
    out = nc.declare_dram_parameter("out", [SS, E], F32, isOutput=True)

    g_part = nc.dram_tensor("g_part", [P, GW], BF16)
    g_full = nc.dram_tensor("g_full", [E, GW], BF16, addr_space="Shared")
    c_part = [nc.dram_tensor(f"c_part{n}", [P, CW], BF16) for n in range(NH)]
    c_full = [
        nc.dram_tensor(f"c_full{n}", [P, CW], BF16, addr_space="Shared")
        for n in range(NH)
    ]

    rg = [list(range(H))]

    with tile.TileContext(nc) as tc:
        with (
            tc.tile_pool(name="singles", bufs=1) as singles,
            tc.tile_pool(name="stat", bufs=4) as stat,
            tc.tile_pool(name="ps_mm", bufs=6, space="PSUM") as ps_mm,
            tc.tile_pool(name="ps_tr", bufs=2, space="PSUM") as ps_tr,
        ):
            ident = singles.tile([P, P], BF16)
            identf = singles.tile([P, P], F32)
            bz8_sb = singles.tile([1, E], F32)
            eps_sb = singles.tile([P, 1], F32)

            with tc.tile_pool(name="pc", bufs=1) as pc:
                c_sb = pc.tile([P, NAT, 512], BF16)
                xts_sb = pc.tile([P, NAT, SS], BF16)
                with tc.tile_pool(name="pa", bufs=1) as pa:
                    a_sb = pa.tile([P, NET, E], BF16)
                    pwqk_cm = tc.tile_pool(name="pwqk", bufs=1)
                    pg_cm = tc.tile_pool(name="pg", bufs=1)
                    pwqk = pwqk_cm.__enter__()
                    pg = pg_cm.__enter__()
                    if True:
                        wqa_sb = pwqk.tile([P, NAT, E], BF16)
                        wka_sb = pwqk.tile([P, NAT, E], BF16)
                        u_sb = pwqk.tile([P, NAT, E], BF16)
                        gx8_sb = pwqk.tile([P, GW], BF16)
                        g_sb = pg.tile([P, NET, GW], BF16)

                        # ===== phase 1: G row-block over full S =====
                        with tc.tile_pool(name="pxa", bufs=1) as pxa:
                            xcol_sb = pxa.tile([P, NKT, P], BF16)
                            xa_sb = pxa.tile([P, NKT, GW], BF16)
                            nc.sync.dma_start(
                                out=xcol_sb,
                                in_=xcp[:, :].rearrange("p (t c) -> p t c", c=P),
                            )
                            XCH = 4
                            for c in range(NKT // XCH):
                                nc.sync.dma_start(
                                    out=xa_sb[:, c * XCH : (c + 1) * XCH, :],
                                    in_=xap[
                                        :, c * XCH * GW : (c + 1) * XCH * GW
                                    ].rearrange("p (t e) -> p t e", e=GW),
                                )
                            make_identity(nc, ident)
                            make_identity(nc, identf)
                            nc.vector.memset(eps_sb, EPS)
                            nc.vector.memset(gx8_sb, 0.0)
                            nc.vector.memset(u_sb[:, NET, :], 0.0)
                            nc.vector.memset(c_sb[:, NET, :], 0.0)

                            gchunks = [(0, 512), (512, 512), (1024, GW - E)]
                            psg = [
                                ps_mm.tile([P, w], F32, tag="mm", name=f"psg_{i}")
                                for i, (o, w) in enumerate(gchunks)
                            ]
                            for k in range(NKT):
                                for i, (o, w) in enumerate(gchunks):
                                    nc.tensor.matmul(
                                        psg[i],
                                        xcol_sb[:, k, :],
                                        xa_sb[:, k, o : o + w],
                                        start=(k == 0),
                                        stop=(k == NKT - 1),
                                    )
                            gp = pxa.tile([P, GW], BF16)
                            for i, (o, w) in enumerate(gchunks):
                                nc.vector.tensor_copy(
                                    out=gp[:, o : o + w], in_=psg[i]
                                )
                            nc.sync.dma_start(out=g_part[:, :], in_=gp)
                            nc.gpsimd.collective_compute(
                                "AllGather",
                                mybir.AluOpType.bypass,
                                replica_groups=rg,
                                ins=[g_part[:, :]],
                                outs=[g_full[:, :]],
                            )
                            # weight loads AFTER the AG trigger: they stream
                            # during the collective instead of delaying it
                            nc.sync.dma_start(
                                out=wqa_sb,
                                in_=wqa[:, :].rearrange("p (t e) -> p t e", e=E),
                            )
                            nc.sync.dma_start(
                                out=wka_sb,
                                in_=wka[:, :].rearrange("p (t e) -> p t e", e=E),
                            )
                            nc.sync.dma_start(
                                out=xts_sb,
                                in_=xts[:, :].rearrange("p (t s) -> p t s", s=SS),
                            )
                            nc.sync.dma_start(out=bz8_sb, in_=rows[0:1, :])

                        # wva/wzT get fresh space freed by the xa pool and
                        # stream during phases 2-3 with no WAR stall
                        pd_cm = tc.tile_pool(name="pd", bufs=1)
                        pwz_cm = tc.tile_pool(name="pwz", bufs=1)
                        pd = pd_cm.__enter__()
                        pwz = pwz_cm.__enter__()
                        d_sb = pd.tile([P, NET, DW], BF16)
                        wva_sb = pwz.tile([P, NET, WVW], BF16)
                        wzT_sb = pwz.tile([P, NET, E], BF16)
                        nc.sync.dma_start(
                            out=wva_sb,
                            in_=wva[:, :].rearrange("p (t e) -> p t e", e=WVW),
                        )
                        nc.sync.dma_start(
                            out=wzT_sb,
                            in_=wzT[:, :].rearrange("p (t e) -> p t e", e=E),
                        )
                        nc.vector.memset(d_sb[:, :, E:DW], 0.0)

                        # ===== gather G per k-tile (pipelines into phase 2);
                        # rebuild the ones-row from the gathered column E =====
                        for t in range(NET):
                            nc.sync.dma_start(
                                out=g_sb[:, t, :],
                                in_=g_full[t * P : (t + 1) * P, :],
                            )
                            pst = ps_tr.tile([1, P], BF16, tag="tr", name="pst")
                            nc.tensor.transpose(
                                pst, g_sb[:, t, E : E + 1], ident
                            )
                            nc.vector.tensor_copy(
                                out=gx8_sb[0:1, t * P : (t + 1) * P], in_=pst
                            )
                        nc.vector.memset(gx8_sb[0:1, E : E + 1], float(S))

                        # ===== phase 2: U = G~ @ Wqa =====
                        def gcols(m):
                            return (m * P, min((m + 1) * P, GW))

                        for (m0, m1) in [(0, 3), (3, 6), (6, 9)]:
                            pss = {}
                            for m in range(m0, m1):
                                for n in range(NH):
                                    pss[m, n] = ps_mm.tile(
                                        [P, 512], F32, tag="mm",
                                        name=f"psu_{m}_{n}",
                                    )
                            for k in range(NAT):
                                for m in range(m0, m1):
                                    c0, c1 = gcols(m)
                                    mw = c1 - c0
                                    lhs = (
                                        g_sb[:, k, c0:c1]
                                        if k < NET
                                        else gx8_sb[:, c0:c1]
                                    )
                                    for n in range(NH):
                                        nc.tensor.matmul(
                                            pss[m, n][0:mw, :],
                                            lhs,
                                            wqa_sb[:, k, n * 512 : (n + 1) * 512],
                                            start=(k == 0),
                                            stop=(k == NAT - 1),
                                        )
                            for m in range(m0, m1):
                                c0, c1 = gcols(m)
                                mw = c1 - c0
                                for n in range(NH):
                                    nc.vector.tensor_copy(
                                        out=u_sb[0:mw, m, n * 512 : (n + 1) * 512],
                                        in_=pss[m, n][0:mw, :],
                                    )

                        # ===== phase 3: scores + softmax (normalized A) =====
                        with tc.tile_pool(name="p3", bufs=3) as p3:
                            for m in range(NET):
                                pss = [
                                    ps_mm.tile([P, 512], F32, tag="mm",
                                               name=f"pssc_{n}")
                                    for n in range(NH)
                                ]
                                for k in range(NAT):
                                    lhs = wka_sb[:, k, m * P : (m + 1) * P]
                                    for n in range(NH):
                                        nc.tensor.matmul(
                                            pss[n], lhs,
                                            u_sb[:, k, n * 512 : (n + 1) * 512],
                                            start=(k == 0), stop=(k == NAT - 1),
                                        )
                                mxs = stat.tile([P, NH], F32, tag="mxs")
                                for n in range(NH):
                                    nc.vector.reduce_max(
                                        out=mxs[:, n : n + 1], in_=pss[n],
                                        axis=mybir.AxisListType.X,
                                    )
                                mx = stat.tile([P, 1], F32, tag="mx")
                                nc.vector.tensor_max(mx, mxs[:, 0:1], mxs[:, 1:2])
                                negmx = stat.tile([P, 1], F32, tag="negmx")
                                nc.vector.tensor_scalar_mul(negmx, mx, -SCALE)
                                a_tmp = p3.tile([P, E], BF16, tag="atmp")
                                rsums = stat.tile([P, NH], F32, tag="rsums")
                                for n in range(NH):
                                    nc.scalar.activation(
                                        out=a_tmp[:, n * 512 : (n + 1) * 512],
                                        in_=pss[n],
                                        func=mybir.ActivationFunctionType.Exp,
                                        bias=negmx, scale=SCALE,
                                        accum_out=rsums[:, n : n + 1],
                                    )
                                rsum = stat.tile([P, 1], F32, tag="rsum")
                                nc.vector.tensor_add(
                                    rsum, rsums[:, 0:1], rsums[:, 1:2]
                                )
                                rcp = stat.tile([P, 1], F32, tag="rcp")
                                nc.vector.reciprocal(out=rcp, in_=rsum)
                                nc.vector.tensor_scalar_mul(
                                    a_sb[:, m, :], a_tmp, rcp
                                )

                    # ===== phase 4a: D = A^T @ [Wv|bv]; 4b: C = D^T @ WzT =====
                    if True:
                        dchunks = [(0, 512), (512, 512), (1024, WVW - E)]
                        for m in range(NET):
                            psd = [
                                ps_mm.tile([P, w], F32, tag="mm", name=f"psd_{i}")
                                for i, (o, w) in enumerate(dchunks)
                            ]
                            for k in range(NET):
                                lhs = a_sb[:, k, m * P : (m + 1) * P]
                                for i, (o, w) in enumerate(dchunks):
                                    nc.tensor.matmul(
                                        psd[i], lhs, wva_sb[:, k, o : o + w],
                                        start=(k == 0), stop=(k == NET - 1),
                                    )
                            for i, (o, w) in enumerate(dchunks):
                                nc.vector.tensor_copy(
                                    out=d_sb[:, m, o : o + w], in_=psd[i]
                                )

                        # 4b: C rows partition-major; m-tile 8 row 0 is r
                        for n in range(NH):
                            for m in range(NAT):
                                ps = ps_mm.tile([P, 512], F32, tag="mm",
                                                name=f"psc_{m}")
                                for k in range(NET):
                                    nc.tensor.matmul(
                                        ps,
                                        d_sb[:, k, m * P : (m + 1) * P],
                                        wzT_sb[:, k, n * 512 : (n + 1) * 512],
                                        start=(k == 0), stop=(k == NET - 1),
                                    )
                                if m < NET:
                                    nc.vector.tensor_copy(
                                        out=c_sb[:, m, :], in_=ps
                                    )
                                else:
                                    nc.vector.tensor_add(
                                        c_sb[0:1, NET, :],
                                        ps[0:1, :],
                                        bz8_sb[0:1, n * 512 : (n + 1) * 512],
                                    )
                            nc.sync.dma_start(
                                out=c_part[n][:, :],
                                in_=c_sb[:, :, :].rearrange("p t c -> p (t c)"),
                            )
                            nc.gpsimd.collective_compute(
                                "AllReduce",
                                mybir.AluOpType.add,
                                replica_groups=rg,
                                ins=[c_part[n][:, :]],
                                outs=[c_full[n][:, :]],
                            )
                        pwz_cm.__exit__(None, None, None)
                        pd_cm.__exit__(None, None, None)
                        pg_cm.__exit__(None, None, None)
                        pwqk_cm.__exit__(None, None, None)

                # ===== phase 5: shard O = xts_aug^T @ Csum; LN1/FFN/LN2 =====
                with tc.tile_pool(name="p5", bufs=1) as p5, \
                     tc.tile_pool(name="p7", bufs=4) as p7:
                    wfT_sb = p5.tile([P, NET, E], BF16)
                    rows_bc = p5.tile([P, 5, E], F32)
                    xs_sb = p5.tile([P, NSS, E], BF16)
                    csum_sb = [
                        p5.tile([P, NAT, 512], BF16, name=f"csum{n}")
                        for n in range(NH)
                    ]
                    nc.sync.dma_start(
                        out=xs_sb, in_=xs[:, :].rearrange("(t p) e -> p t e", p=P)
                    )
                    for k in range(5):
                        nc.sync.dma_start(
                            out=rows_bc[:, k, :], in_=_bcast_row(rows[k + 1 : k + 2, :])
                        )
                    nc.sync.dma_start(
                        out=wfT_sb,
                        in_=wfT[:, :].rearrange("p (t e) -> p t e", e=E),
                    )
                    o_sb = p5.tile([P, NSS, E], F32)
                    ln1_sb = p5.tile([P, NSS, E], F32)
                    l1t_sb = p5.tile([P, NET, SS], BF16)

                    bsts = [
                        stat.tile([P, 2, 6], F32, tag="bst", name=f"bst1_{st}")
                        for st in range(NSS)
                    ]
                    for n in range(1):
                        # 3-tile chunks: the O k-loop starts on chunk 0 while
                        # the rest of Csum streams in behind it
                        for t0 in range(0, NAT, 3):
                            nc.sync.dma_start(
                                out=csum_sb[n][:, t0 : t0 + 3, :],
                                in_=c_full[n][
                                    :, t0 * 512 : (t0 + 3) * 512
                                ].rearrange("p (t c) -> p t c", c=512),
                            )
                        for m in range(NSS):
                            ps = ps_mm.tile([P, 512], F32, tag="mm",
                                            name=f"pso_{m}")
                            for k in range(NAT):
                                nc.tensor.matmul(
                                    ps,
                                    xts_sb[:, k, m * P : (m + 1) * P],
                                    csum_sb[n][:, k, :],
                                    start=(k == 0), stop=(k == NAT - 1),
                                )
                            # stats straight off PSUM: the LN1 chain is ready
                            # the moment the last O tile's copy lands
                            nc.vector.bn_stats(out=bsts[m][:, n, :], in_=ps)
                            nc.scalar.copy(
                                out=o_sb[:, m, n * 512 : (n + 1) * 512], in_=ps
                            )

                    def ln_apply(dst, src, mv, rstd, r_g, r_b, skip_gb):
                        nc.vector.tensor_scalar(
                            out=dst, in0=src, scalar1=mv[:, 0:1], scalar2=rstd,
                            op0=mybir.AluOpType.subtract, op1=mybir.AluOpType.mult,
                        )
                        if not skip_gb:
                            nc.vector.tensor_mul(dst, dst, rows_bc[:, r_g, :])
                            nc.vector.tensor_add(dst, dst, rows_bc[:, r_b, :])

                    # LN1: batched stats -> one sqrt + one reciprocal for all 4
                    # tiles, then per-tile normalize + residual + transposes
                    mv1 = stat.tile([P, NSS, 2], F32, tag="mv1")
                    sd1 = stat.tile([P, NSS], F32, tag="sd1")
                    rstd1 = stat.tile([P, NSS], F32, tag="rstd1")
                    # n=1 pass: each tile's O matmul is followed immediately
                    # by its full LN1/transpose/FFN/LN2 chain, so tile st's
                    # tail hides behind tile st+1's O matmuls
                    for t0 in range(0, NAT, 3):
                        nc.sync.dma_start(
                            out=csum_sb[1][:, t0 : t0 + 3, :],
                            in_=c_full[1][
                                :, t0 * 512 : (t0 + 3) * 512
                            ].rearrange("p (t c) -> p t c", c=512),
                        )
                    for st in range(NSS):
                        ps = ps_mm.tile([P, 512], F32, tag="mm",
                                        name=f"pso1_{st}")
                        for k in range(NAT):
                            nc.tensor.matmul(
                                ps,
                                xts_sb[:, k, st * P : (st + 1) * P],
                                csum_sb[1][:, k, :],
                                start=(k == 0), stop=(k == NAT - 1),
                            )
                        nc.vector.bn_stats(out=bsts[st][:, 1, :], in_=ps)
                        nc.scalar.copy(
                            out=o_sb[:, st, 512:E], in_=ps
                        )
                        nc.vector.bn_aggr(out=mv1[:, st, :], in_=bsts[st])
                        nc.scalar.activation(
                            out=sd1[:, st : st + 1], in_=mv1[:, st, 1:2],
                            func=mybir.ActivationFunctionType.Sqrt,
                            bias=eps_sb[:, :],
                        )
                        nc.vector.reciprocal(
                            out=rstd1[:, st : st + 1], in_=sd1[:, st : st + 1]
                        )
                        t1 = ln1_sb[:, st, :]
                        ln_apply(
                            t1, o_sb[:, st, :], mv1[:, st, :],
                            rstd1[:, st : st + 1], L_G1, L_B1, id_g1b1,
                        )
                        nc.vector.tensor_add(t1, t1, xs_sb[:, st, :])
                        for eb in range(NET):
                            pstf = ps_tr.tile([P, P], F32, tag="tr", name="pstf")
                            nc.tensor.transpose(
                                pstf, ln1_sb[:, st, eb * P : (eb + 1) * P], identf
                            )
                            nc.scalar.copy(
                                out=l1t_sb[:, eb, st * P : (st + 1) * P], in_=pstf
                            )
                        # FFN -> LN2 -> out for this tile queues right behind
                        # its own transposes on the tensor engine
                        f1 = p7.tile([P, E], F32, tag="f1", name=f"f1_{st}")
                        bst2 = stat.tile([P, 2, 6], F32, tag="bst2",
                                         name=f"bst2_{st}")
                        for n in range(NH):
                            ps = ps_mm.tile([P, 512], F32, tag="mm",
                                            name=f"psf_{n}")
                            for k in range(NET):
                                nc.tensor.matmul(
                                    ps,
                                    l1t_sb[:, k, st * P : (st + 1) * P],
                                    wfT_sb[:, k, n * 512 : (n + 1) * 512],
                                    start=(k == 0), stop=(k == NET - 1),
                                )
                            nc.vector.tensor_add(
                                f1[:, n * 512 : (n + 1) * 512],
                                ps,
                                rows_bc[:, L_BF, n * 512 : (n + 1) * 512],
                            )
                            nc.vector.bn_stats(
                                out=bst2[:, n, :],
                                in_=f1[:, n * 512 : (n + 1) * 512],
                            )
                        mv2 = stat.tile([P, 2], F32, tag="mv2")
                        nc.vector.bn_aggr(out=mv2, in_=bst2)
                        sd2 = stat.tile([P, 1], F32, tag="sd2")
                        nc.scalar.activation(
                            out=sd2, in_=mv2[:, 1:2],
                            func=mybir.ActivationFunctionType.Sqrt,
                            bias=eps_sb[:, :],
                        )
                        rstd2 = stat.tile([P, 1], F32, tag="rstd2")
                        nc.vector.reciprocal(out=rstd2, in_=sd2)
                        fo = p7.tile([P, E], F32, tag="fo", name=f"fo_{st}")
                        ln_apply(fo, f1, mv2, rstd2, L_G2, L_B2, id_g2b2)
                        nc.vector.tensor_add(fo, fo, ln1_sb[:, st, :])
                        nc.sync.dma_start(out=out[st * P : (st + 1) * P, :], in_=fo)

    nc.finalize()
    return nc


_NC_CACHE = None


def _ptile(a, p=P):
    T = a.shape[0] // p
    return np.ascontiguousarray(
        a.reshape(T, p, a.shape[1]).transpose(1, 0, 2).reshape(p, T * a.shape[1])
    )


def kernel(**inputs) -> np.ndarray:
    global _NC_CACHE, LAST_RESULT
    x = np.asarray(inputs["x"], np.float32)
    Wq = np.asarray(inputs["Wq"], np.float32)
    bq = np.asarray(inputs["bq"], np.float32)
    Wk = np.asarray(inputs["Wk"], np.float32)
    bk = np.asarray(inputs["bk"], np.float32)
    Wv = np.asarray(inputs["Wv"], np.float32)
    bv = np.asarray(inputs["bv"], np.float32)
    Wz = np.asarray(inputs["Wz"], np.float32)
    bz = np.asarray(inputs["bz"], np.float32)
    g1 = np.asarray(inputs["g1"], np.float32)
    b1 = np.asarray(inputs["b1"], np.float32)
    Wf = np.asarray(inputs["Wf"], np.float32)
    bf_ = np.asarray(inputs["bf"], np.float32)
    g2 = np.asarray(inputs["g2"], np.float32)
    b2 = np.asarray(inputs["b2"], np.float32)

    BF = ml_dtypes.bfloat16
    id_g1b1 = bool(np.all(g1 == 1.0) and np.all(b1 == 0.0))
    id_g2b2 = bool(np.all(g2 == 1.0) and np.all(b2 == 0.0))
    key = (id_g1b1, id_g2b2)
    if _NC_CACHE is None or _NC_CACHE[0] != key:
        _NC_CACHE = (key, build_nc(id_g1b1, id_g2b2))
    nc = _NC_CACHE[1]

    xa_np = np.concatenate(
        [x, np.ones((S, 1), np.float32), np.zeros((S, GW - E - 1), np.float32)],
        axis=1,
    ).astype(BF)
    xap_np = _ptile(xa_np)
    xt_np = np.ascontiguousarray(x.T)
    wfT_np = _ptile(np.ascontiguousarray(Wf.T).astype(BF))
    rows_np = np.ascontiguousarray(
        np.stack([bz / H, g1, b1, bf_, g2, b2]).astype(np.float32)
    )
    pad_w = np.zeros((EA - E - 1, E), np.float32)

    in_maps = []
    for h in range(H):
        wqa_h = np.concatenate([Wq[h].T, bq[h][None, :], pad_w], axis=0).astype(BF)
        wka_h = np.concatenate([Wk[h].T, bk[h][None, :], pad_w], axis=0).astype(BF)
        wva_h = np.concatenate(
            [Wv[h], bv[h][:, None], np.zeros((E, WVW - E - 1), np.float32)], axis=1
        ).astype(BF)
        wzT_h = np.ascontiguousarray(Wz[:, h * E : (h + 1) * E].T).astype(BF)
        xcp_h = _ptile(np.ascontiguousarray(xa_np[:, h * P : (h + 1) * P]))
        xts_h = np.concatenate(
            [
                xt_np[:, h * SS : (h + 1) * SS],
                np.ones((1, SS), np.float32),
                np.zeros((EA - E - 1, SS), np.float32),
            ],
            axis=0,
        ).astype(BF)
        in_maps.append(
            {
                "xap": xap_np,
                "xcp": xcp_h,
                "wqa": _ptile(wqa_h),
                "wka": _ptile(wka_h),
                "wva": _ptile(wva_h),
                "wzT": _ptile(wzT_h),
                "wfT": wfT_np,
                "xts": _ptile(xts_h),
                "xs": np.ascontiguousarray(x[h * SS : (h + 1) * SS]).astype(BF),
                "rows": rows_np,
            }
        )

    res = run_bass_kernel_spmd(nc, in_maps, list(range(H)))
    LAST_RESULT = res
    out = np.empty((S, E), np.float32)
    for h in range(H):
        out[h * SS : (h + 1) * SS] = res.results[h]["out"]
    return out
